# revision 49
# baseline (speedup 1.0000x reference)
"""Bahdanau-style attention kernel for Trainium2, 8 NeuronCores, data-parallel over
batch, with mask-sparsity: masked positions (mask==1) contribute exactly 0 to the
softmax, so their rows of encoder_outputs are never computed.

Reference computation, per (b, s):
    energy = tanh(dec @ Wd + enc @ We + b_attn)          # [B,S,H]
    att    = energy @ v_w                                 # [B,S]
    att    = where(mask==1, -1e10, att)
    out    = softmax(att, axis=1)

Full shapes: B=64, S=2048, H=1024. Each core takes 8 batches.

Host-side prep (data movement only): live rows (mask==0) of encoder_outputs are
compacted, cast to fp16, and transposed to [128 h-partitions, HB, R] per batch;
weights/vectors are pre-cast to fp16 in the on-chip layouts. The kernel returns
compact per-row probabilities which the host scatters back to [B, S].

Device pipeline, per batch (PE fp16, f32 accumulation; rows on PSUM partitions,
kout on the free axis so nothing but the main matmuls ever touches the PE):
  - one contiguous DMA pulls encT [128, HB*R] fp16 into SBUF (double-buffered).
  - per 128-row block: psum[rows, kout] += encT[h, rows].T @ We[h, kout],
    8 h-blocks x 2 kout-halves of 512.
  - DVE adds bias row (dec@Wd + b_attn, computed once on the PE and
    partition-broadcast via a DRAM bounce); ACT applies tanh -> fp16.
  - the v_w dot is one fused gpsimd scalar_tensor_tensor (mult + row-sum
    accumulator) -> att[rows, 1] per block.
  - ACT exp, DVE pad-mask multiply + free-axis reduce, gpsimd all-reduces Z
    across partitions, DVE reciprocal + scale, compact probabilities DMA out.
"""
import numpy as np

B, S, H = 64, 2048, 1024
NCORES = 8
BPC = B // NCORES          # batches per core
HB = H // 128              # h blocks (contraction)
NKH = 2                    # kout halves (512 each, one PSUM bank per half)
KH = H // NKH
R_DEFAULT = 1152           # padded live rows per batch (multiple of 128)

_graph_cache = {}


def _build(R=R_DEFAULT):
    import concourse.bass as bass
    import concourse.bacc as bacc
    import concourse.tile as tile
    from concourse import mybir
    from concourse import bass_isa

    F32 = mybir.dt.float32
    F16 = mybir.dt.float16
    AF = mybir.ActivationFunctionType
    ALU = mybir.AluOpType
    RB = R // 128

    nc = bacc.Bacc(trn_type="TRN2", target_bir_lowering=False)

    enct_ext = nc.declare_dram_parameter("encT", [BPC, 128, HB * R], F16, isOutput=False)
    we_ext = nc.declare_dram_parameter("we", [128, HB * H], F16, isOutput=False)
    wd_ext = nc.declare_dram_parameter("wd", [128, HB * H], F16, isOutput=False)
    dect_ext = nc.declare_dram_parameter("dect", [128, HB * BPC], F16, isOutput=False)
    brow_ext = nc.declare_dram_parameter("brow", [1, H], F16, isOutput=False)
    ones_ext = nc.declare_dram_parameter("ones1", [1, BPC], F16, isOutput=False)
    vrep_ext = nc.declare_dram_parameter("vrep", [128, H], F16, isOutput=False)
    kc_ext = nc.declare_dram_parameter("kc", [BPC, 128, RB], F32, isOutput=False)
    out_ext = nc.declare_dram_parameter("out", [BPC, 128, RB], F32, isOutput=True)

    bias_dram = nc.dram_tensor("bias_dram", [BPC, NKH, KH], F32)

    with tile.TileContext(nc) as tc:
        with (
            tc.tile_pool(name="weights", bufs=1) as wpool,
            tc.tile_pool(name="enct", bufs=2) as tpool,
            tc.tile_pool(name="biasb", bufs=2) as bpool,
            tc.tile_pool(name="esum", bufs=2) as epool,
            tc.tile_pool(name="energy", bufs=2) as engpool,
            tc.tile_pool(name="rows", bufs=2) as rpool,
            tc.tile_pool(name="psum_mm", bufs=3, space="PSUM") as psum_pool,
            tc.tile_pool(name="psum_setup", bufs=1, space="PSUM") as spool,
        ):
            # ---------------- setup ----------------
            # Strict priority order on the sync DMA queue: wd16 first (the
            # bias matmuls are the PE's first work while encT[0] streams in),
            # then the first kout-half of We, encT[0], the second half, and
            # encT[1..] behind.
            we16 = wpool.tile([128, NKH, HB, KH], F16, tag="we")
            vrep = wpool.tile([128, H], F16, tag="vrep")
            bias_all = wpool.tile([BPC, H], F32, tag="bias_all")
            nc.scalar.dma_start(out=vrep[:], in_=vrep_ext[:])

            def emit_bias_setup():
                # bias_all[b, k] = (dec @ Wd)[b, k] + b_attn[k]; all on PE+ACT
                # so no DVE-queue ordering hazard with the per-block bias adds.
                for h in range(NKH):
                    ps = spool.tile([BPC, KH], F32, tag="psetup")
                    for hb in range(HB):
                        nc.tensor.matmul(
                            ps[:], dect[:, hb, :], wd16[:, hb, h * KH : (h + 1) * KH],
                            start=(hb == 0), stop=False,
                        )
                    nc.tensor.matmul(
                        ps[:], ones1[:], brow[:, h * KH : (h + 1) * KH],
                        start=False, stop=True,
                    )
                    nc.scalar.activation(bias_all[:, h * KH : (h + 1) * KH], ps[:], AF.Copy)
                    nc.scalar.dma_start(
                        out=bias_dram[:, h, :], in_=bias_all[:, h * KH : (h + 1) * KH]
                    )

            # ---------------- per-batch loads ----------------
            enct_tiles, kc_tiles, bias_tiles = {}, {}, {}

            def load_enct(b, queue):
                t = tpool.tile([128, HB, R], F16, tag="enct")
                queue.dma_start(out=t[:].rearrange("p hb r -> p (hb r)"), in_=enct_ext[b])
                enct_tiles[b] = t

            def load_meta(b):
                kc = rpool.tile([128, RB], F32, tag="kc")
                nc.scalar.dma_start(out=kc[:], in_=kc_ext[b])
                kc_tiles[b] = kc

            def load_bias(b):
                bb = bpool.tile([128, NKH, KH], F32, tag="biasb")
                for h, q in ((0, nc.scalar), (1, nc.sync)):
                    q.dma_start(
                        out=bb[:, h, :],
                        in_=bias_dram[b : b + 1, h, :].broadcast_to([128, KH]),
                    )
                bias_tiles[b] = bb

            # ---------------- per-batch compute ----------------
            def emit_batch(b):
                enct = enct_tiles.pop(b)
                kc = kc_tiles.pop(b)
                bb = bias_tiles.pop(b)
                att = rpool.tile([128, RB], F32, tag="att")
                for rb in range(RB):
                    pks = []
                    for h in range(NKH):
                        pk = psum_pool.tile([128, KH], F32, tag="pmm")
                        for hb in range(HB):
                            nc.tensor.matmul(
                                pk[:],
                                enct[:, hb, rb * 128 : (rb + 1) * 128],
                                we16[:, h, hb, :],
                                start=(hb == 0), stop=(hb == HB - 1),
                            )
                        pks.append(pk)
                    esum = epool.tile([128, NKH, KH], F32, tag="esum")
                    for h in range(NKH):
                        nc.vector.tensor_tensor(
                            esum[:, h, :], pks[h][:], bb[:, h, :], ALU.add
                        )
                    eng = engpool.tile([128, NKH, KH], F16, tag="energy")
                    nc.scalar.activation(
                        eng[:].rearrange("p a k -> p (a k)"),
                        esum[:].rearrange("p a k -> p (a k)"),
                        AF.Tanh,
                    )
                    # fused v_w dot: prod = eng * vrep, att[:, rb] = sum(prod)
                    prod = engpool.tile([128, NKH, KH], F16, tag="prod")
                    nc.vector.scalar_tensor_tensor(
                        out=prod[:].rearrange("p a k -> p (a k)"),
                        in0=eng[:].rearrange("p a k -> p (a k)"),
                        scalar=0.0,
                        in1=vrep[:],
                        op0=ALU.bypass,
                        op1=ALU.mult,
                        accum_out=att[:, rb : rb + 1],
                    )
                # softmax over live rows (pads have kc=0)
                e = rpool.tile([128, RB], F32, tag="e")
                nc.scalar.activation(e[:], att[:], AF.Exp)
                ec = rpool.tile([128, RB], F32, tag="ec")
                nc.vector.tensor_tensor(ec[:], e[:], kc[:], ALU.mult)
                zcol = rpool.tile([128, 1], F32, tag="zcol")
                nc.vector.tensor_reduce(zcol[:], ec[:], mybir.AxisListType.X, ALU.add)
                zall = rpool.tile([128, 1], F32, tag="zall")
                nc.gpsimd.partition_all_reduce(zall[:], zcol[:], 128, bass_isa.ReduceOp.add)
                zr = rpool.tile([128, 1], F32, tag="zr")
                nc.vector.reciprocal(zr[:], zall[:])
                probs = rpool.tile([128, RB], F32, tag="probs")
                nc.vector.tensor_scalar(probs[:], ec[:], zr[:], None, ALU.mult)
                nc.gpsimd.dma_start(out=out_ext[b], in_=probs[:])

            setup_stack = tc.tile_pool(name="wsetup", bufs=1)
            wsetup = setup_stack.__enter__()
            wd16 = wsetup.tile([128, HB, H], F16, tag="wd")
            dect = wsetup.tile([128, HB, BPC], F16, tag="dect")
            brow = wsetup.tile([1, H], F16, tag="brow")
            ones1 = wsetup.tile([1, BPC], F16, tag="ones1")
            nc.scalar.dma_start(out=dect[:].rearrange("p hb b -> p (hb b)"), in_=dect_ext[:])
            nc.scalar.dma_start(out=brow[:], in_=brow_ext[:])
            nc.scalar.dma_start(out=ones1[:], in_=ones_ext[:])
            nc.sync.dma_start(out=wd16[:].rearrange("p hb k -> p (hb k)"), in_=wd_ext[:])
            nc.sync.dma_start(
                out=we16[:, 0].rearrange("p hb k -> p (hb k)"), in_=we_ext[:, : HB * KH]
            )
            load_meta(0)
            load_enct(0, nc.sync)
            nc.sync.dma_start(
                out=we16[:, 1].rearrange("p hb k -> p (hb k)"), in_=we_ext[:, HB * KH :]
            )
            load_meta(1)
            emit_bias_setup()
            load_bias(0)
            load_bias(1)

            for b in range(BPC):
                emit_batch(b)
                if b == 0:
                    load_enct(1, nc.sync)
                if b + 2 < BPC:
                    load_meta(b + 2)
                    load_enct(b + 2, nc.sync)
                    load_bias(b + 2)
                if b == 0:
                    setup_stack.__exit__(None, None, None)

    nc.compile()
    return nc


def _get_graph(R=R_DEFAULT):
    if R not in _graph_cache:
        _graph_cache[R] = _build(R)
    return _graph_cache[R]


def _prep(enc, msk):
    """Host-side data movement: per-batch compaction + fp16 cast + transpose."""
    counts = (msk == 0).sum(axis=1)
    R = max(R_DEFAULT, int(-(-counts.max() // 128) * 128))
    RB = R // 128

    encT = np.zeros((NCORES, BPC, 128, HB * R), np.float16)
    kc = np.zeros((NCORES, BPC, 128, RB), np.float32)
    idxs = []
    for ci in range(NCORES):
        row = []
        for b in range(BPC):
            idx = np.where(msk[ci * BPC + b] == 0)[0]
            n = len(idx)
            comp = np.zeros((R, H), np.float16)
            comp[:n] = enc[ci * BPC + b, idx, :]
            # [R, H] -> [H, R] -> [HB, 128, R] -> [128, HB, R]
            t = comp.T.reshape(HB, 128, R).transpose(1, 0, 2)
            encT[ci, b] = t.reshape(128, HB * R)
            # row r = rb*128 + p lives at kc[p, rb]
            live = np.zeros(R, np.float32)
            live[:n] = 1.0
            kc[ci, b] = live.reshape(RB, 128).T
            row.append(idx)
        idxs.append(row)
    return R, encT, kc, idxs


def _run(decoder_hidden, encoder_outputs, mask, W_attn, b_attn, v_w, **spmd_kwargs):
    from concourse.bass_utils import run_bass_kernel_spmd

    dec = np.asarray(decoder_hidden, dtype=np.float32)
    enc = np.asarray(encoder_outputs, dtype=np.float32)
    msk = np.asarray(mask, dtype=np.int32)
    W = np.asarray(W_attn, dtype=np.float32)
    bb = np.asarray(b_attn, dtype=np.float32)
    vv = np.asarray(v_w, dtype=np.float32)

    R, encT, kc, idxs = _prep(enc, msk)
    nc = _get_graph(R)

    # weight/vector payloads in on-chip layouts (pure data movement)
    we16 = (
        W[H:].astype(np.float16)
        .reshape(HB, 128, NKH, KH).transpose(1, 2, 0, 3).reshape(128, -1)
    )
    wd16 = W[:H].astype(np.float16).reshape(HB, 128, H).transpose(1, 0, 2).reshape(128, -1)
    vrep = np.ascontiguousarray(np.broadcast_to(vv.astype(np.float16), (128, H)))
    brow = bb.astype(np.float16).reshape(1, H)
    ones1 = np.ones((1, BPC), np.float16)

    in_maps = []
    for i in range(NCORES):
        sl = slice(i * BPC, (i + 1) * BPC)
        dect = dec[sl].T.astype(np.float16).reshape(HB, 128, BPC).transpose(1, 0, 2).reshape(128, -1)
        in_maps.append(
            {
                "encT": encT[i],
                "we": np.ascontiguousarray(we16),
                "wd": np.ascontiguousarray(wd16),
                "dect": np.ascontiguousarray(dect),
                "brow": brow,
                "ones1": ones1,
                "vrep": vrep,
                "kc": kc[i],
            }
        )
    res = run_bass_kernel_spmd(nc, in_maps, core_ids=list(range(NCORES)), **spmd_kwargs)
    out = np.zeros((B, S), np.float32)
    for ci in range(NCORES):
        for b in range(BPC):
            idx = idxs[ci][b]
            # out[b] is [128, RB]; row r = rb*128+p -> transpose then flatten
            flat = res.results[ci]["out"][b].T.reshape(-1)
            out[ci * BPC + b, idx] = flat[: len(idx)]
    return out, res


def kernel(decoder_hidden, encoder_outputs, mask, W_attn, b_attn, v_w):
    out, _ = _run(decoder_hidden, encoder_outputs, mask, W_attn, b_attn, v_w)
    return out
